# revision 15
# baseline (speedup 1.0000x reference)
"""Gated MSA-row attention (AlphaFold-style) Trainium2 kernel.

Sharding: data-parallel over the 128 MSA rows -> 16 rows/core on 8 cores;
rows processed in pairs, software-pipelined across pairs.

Structure (vs the 148us baseline):
- bias1 rides the ACT exp bias (per-partition [128,1], k-major layout)
  -> no eb1 v-scaling / eb1w weights; denom weights = constant 2.0
- lt PSUM [128,1024] x bufs=2, the two head-groups' row-packed matmuls
  interleaved so concurrent drains hit 4 distinct banks (same-bank
  concurrent row-tile drains are fatal on HW)
- exp per (r,hg,kc): contiguous E writes, per-kc bias column
- hadamard split per (r,hg,kc) -> denominators start at half-hadamard
- bc/ot/fin share one 2-buffer PSUM ring; denoms+recips for both rows
  run before AV so ring turnover never waits on late consumers
- gating: f16 tmp, merged [128,512] ops, gt in one tile (r-major)
- PSUM evacs split DVE/ACT by pair parity
- software pipeline: proj(p+1) is emitted between logits(p) and the
  back half of p, filling the PE while ACT runs the exp chain
"""

import math
import sys

sys.path.insert(0, "/opt/trn_rl_repo")

import numpy as np

import concourse.bass as bass
import concourse.mybir as mybir
from concourse import bacc
from concourse.tile import TileContext

F32 = mybir.dt.float32
F16 = mybir.dt.float16

H = 8
CH = 32
CQ = 256
Q = 256
K = 256
S = 128
NCORES = 8
RPC = S // NCORES
PAIRS = RPC // 2

Exp = mybir.ActivationFunctionType.Exp
Tanh = mybir.ActivationFunctionType.Tanh
MULT = mybir.AluOpType.mult
ADD = mybir.AluOpType.add


def _emit_proj(nc, tc, P, p):
    """Input DMA + all four projections + their PSUM evacuations."""
    row = [p * 2, p * 2 + 1]
    ps_kv = P["pskv"]
    pe = p % 2
    evac = [nc.vector.tensor_copy, nc.scalar.copy]

    x = P["xin"].tile([128, 2048], F16, tag="x", name="x")
    for b in range(4):
        nc.gpsimd.dma_start(x[:, b * 512:(b + 1) * 512],
                            P["x_d"][p][:, b * 512:(b + 1) * 512])
    kv = [x[:, 512 * c:512 * (c + 1)] for c in range(2)]
    qx = [x[:, 1024 + 512 * c:1536 + 512 * c] for c in range(2)]

    kt_sb, qt_sb = [], []
    for m in range(2):
        msl = slice(m * 128, (m + 1) * 128)
        with nc.named_scope("ktproj"):
            pk = ps_kv.tile([128, 512], F32, tag="kv", name="pkv")
            nc.tensor.matmul(pk[:], P["wk_sb"][0][:, msl], kv[0],
                             start=True, stop=False)
            nc.tensor.matmul(pk[:], P["wk_sb"][1][:, msl], kv[1],
                             start=False, stop=True)
            kt = P["kt"].tile([128, 512], F16, tag=f"kt{m}", name=f"kt{m}")
            evac[(pe + m) % 2](kt[:], pk[:])
            kt_sb.append(kt)
        with nc.named_scope("qproj"):
            pq = ps_kv.tile([128, 512], F32, tag="kv", name="pkv")
            nc.tensor.matmul(pq[:], P["wq_sb"][0][:, msl], qx[0],
                             start=True, stop=False)
            nc.tensor.matmul(pq[:], P["wq_sb"][1][:, msl], qx[1],
                             start=False, stop=True)
            qt = P["qt"].tile([128, 512], F16, tag=f"qt{m}", name=f"qt{m}")
            evac[(pe + m + 1) % 2](qt[:], pq[:])
            qt_sb.append(qt)

    v_sb = []
    for r in range(2):
        with nc.named_scope("vproj"):
            pv = ps_kv.tile([128, 512], F32, tag="kv", name="pkv")
            for kc in range(2):
                for c in range(2):
                    nc.tensor.matmul(
                        pv[:, kc * 256:(kc + 1) * 256],
                        kv[c][:, r * 256 + kc * 128:r * 256 + kc * 128 + 128],
                        P["wv_sb"][c][:],
                        start=(c == 0), stop=(c == 1))
            vt = P["vt"].tile([128, 512], F16, tag=f"v{r}", name=f"v{r}")
            evac[(pe + r) % 2](vt[:], pv[:])
            v_sb.append(vt)

    # gate projection -> tanh(x/2); gt layout [128, (r 2, m 2, q 256)]
    gt = P["gt"].tile([128, 1024], F16, tag="gt", name="gt")
    go = gt[:].rearrange("p (r m q) -> p r m q", r=2, m=2)
    for m in range(2):
        msl = slice(m * 128, (m + 1) * 128)
        with nc.named_scope("gproj"):
            pg = ps_kv.tile([128, 512], F32, tag="kv", name="pkv")
            nc.tensor.matmul(pg[:], P["wg_sb"][0][:, msl], qx[0],
                             start=True, stop=False)
            nc.tensor.matmul(pg[:], P["wg_sb"][1][:, msl], qx[1],
                             start=False, stop=True)
            nc.scalar.activation(go[:, :, m, :], pg[:], Tanh, scale=0.5)

    return {"row": row, "kt_sb": kt_sb, "qt_sb": qt_sb, "v_sb": v_sb,
            "gt": gt, "pe": pe}


def _emit_attn(nc, tc, P, p, ctx):
    """Logits + exp + hadamard."""
    row = ctx["row"]
    kt_sb, qt_sb = ctx["kt_sb"], ctx["qt_sb"]
    ps_lt = P["pslt"]

    e_sb = P["esb"].tile([128, 8192], F16, tag="e", name="e")
    for r in range(2):
        for hg in range(2):
            with nc.named_scope("logits"):
                # head hp owns PSUM bank hp of lt; kc0/kc1 in its halves
                lt = ps_lt.tile([128, 2048], F32, tag="lt", name="lt")
                for kc in range(2):
                    for hp in range(4):
                        st = 32 * hp
                        nc.tensor.matmul(
                            lt[:, hp * 512 + kc * 256:
                               hp * 512 + kc * 256 + 256],
                            kt_sb[hg][st:st + 32,
                                      r * 256 + kc * 128:
                                      r * 256 + kc * 128 + 128],
                            qt_sb[hg][st:st + 32, r * 256:r * 256 + 256],
                            start=True, stop=True,
                            tile_position=(st, 0))
            with nc.named_scope("exp"):
                blk = hg * 2 + r
                lv = lt[:].rearrange("p (hp t) -> p hp t", hp=4)
                for kc in range(2):
                    col = row[r] * 2 + kc
                    nc.scalar.activation(
                        e_sb[:, kc * 4096 + blk * 1024:
                             kc * 4096 + blk * 1024 + 1024],
                        lv[:, :, kc * 256:(kc + 1) * 256], Exp,
                        bias=P["b1_sb"][:, col:col + 1])

    a_sb = []
    for kc in range(2):
        at = P["asb"].tile([128, 4096], F16, tag=f"a{kc}", name=f"a{kc}")
        a_sb.append(at)
    for r in range(2):
        for hg in range(2):
            for kc in range(2):
                with nc.named_scope("hadamard"):
                    cs = slice(hg * 2048 + r * 1024,
                               hg * 2048 + r * 1024 + 1024)
                    nc.vector.tensor_tensor(
                        a_sb[kc][:, cs],
                        e_sb[:, kc * 4096 + (hg * 2 + r) * 1024:
                             kc * 4096 + (hg * 2 + r) * 1024 + 1024],
                        P["eb2_sb"][kc][:, cs], MULT)
    ctx["a_sb"] = a_sb


def _emit_back(nc, tc, P, p, ctx):
    row, a_sb = ctx["row"], ctx["a_sb"]
    v_sb, gt, pe = ctx["v_sb"], ctx["gt"], ctx["pe"]
    ps_obf = P["psobf"]
    evac = [nc.vector.tensor_copy, nc.scalar.copy]

    # denominators + reciprocals for both rows first (fast ring turnover)
    rc_sb = []
    for r in range(2):
        bc = ps_obf.tile([128, 512], F32, tag="obf", name="bc")
        with nc.named_scope("denom"):
            for hp in range(4):
                for kc in range(2):
                    av = a_sb[kc][:].rearrange("p (hg b) -> p hg b", hg=2)
                    rhs = av[:, :, r * 1024 + hp * 256:
                             r * 1024 + hp * 256 + 256]
                    nc.tensor.matmul(
                        bc[32 * hp:32 * hp + 32, :],
                        P["two_sb"][:, 0:32], rhs,
                        start=(kc == 0), stop=(kc == 1),
                        tile_position=(0, 32 * hp))
        rc = P["rcp"].tile([128, 512], F32, tag=f"rc{r}", name=f"rc{r}")
        with nc.named_scope("recip"):
            nc.vector.reciprocal_approx_fast(rc[:], bc[:])
        rc_sb.append(rc)

    ot_sb = []
    for r in range(2):
        ot = ps_obf.tile([128, 512], F32, tag="obf", name="ot")
        ot_sb.append(ot)
        with nc.named_scope("av"):
            for hg in range(2):
                for hp in range(4):
                    for kc in range(2):
                        off = (hg * 2 + r) * 1024 + hp * 256
                        nc.tensor.matmul(
                            ot[32 * hp:32 * hp + 32,
                               hg * 256:hg * 256 + 256],
                            v_sb[r][:, kc * 256 + hg * 128 + 32 * hp:
                                    kc * 256 + hg * 128 + 32 * hp + 32],
                            a_sb[kc][:, off:off + 256],
                            start=(kc == 0), stop=(kc == 1),
                            tile_position=(0, 32 * hp))

    # gating: og = (1 + tanh) * ot * rc  (the 1/2s live in two_sb)
    og_sb = []
    for r in range(2):
        with nc.named_scope("gating"):
            tmp = P["osb"].tile([128, 512], F16, tag="gtmp", name="gtmp")
            nc.vector.scalar_tensor_tensor(
                tmp[:], gt[:, r * 512:(r + 1) * 512], 1.0, ot_sb[r][:],
                ADD, MULT)
            og = P["otg"].tile([128, 512], F16, tag=f"og{r}", name=f"og{r}")
            nc.vector.tensor_tensor(og[:], tmp[:], rc_sb[r][:], MULT)
            og_sb.append(og)

    ob = P["osb"].tile([128, 1024], F32, tag="ob", name="ob")
    for r in range(2):
        fin = ps_obf.tile([128, 512], F32, tag="obf", name="fin")
        with nc.named_scope("outproj"):
            for qc in range(2):
                for hg in range(2):
                    nc.tensor.matmul(
                        fin[:, qc * 256:qc * 256 + 256],
                        og_sb[r][:, hg * 256 + qc * 128:
                                 hg * 256 + qc * 128 + 128],
                        P["wo_sb"][hg][:],
                        start=(hg == 0), stop=(hg == 1))
        with nc.named_scope("outevac"):
            evac[(pe + r) % 2](ob[:, r * 512:(r + 1) * 512], fin[:])
    nc.sync.dma_start(
        P["out_d"][row[0]:row[0] + 2].rearrange(
            "r (qc p) d -> p r qc d", qc=2),
        ob[:].rearrange("p (r qc d) -> p r qc d", r=2, qc=2))


def build_nc():
    nc = bacc.Bacc("TRN2", target_bir_lowering=False)

    P = {}
    P["x_d"] = nc.dram_tensor("xin", [PAIRS, 128, 2048], F16,
                              kind="ExternalInput")
    wd = {nm: nc.dram_tensor(f"w{nm}t", [CQ, 256], F16, kind="ExternalInput")
          for nm in ("q", "k", "v", "g", "o")}
    b1_d = nc.dram_tensor("b1s", [128, 2 * RPC], F32, kind="ExternalInput")
    eb2_d = nc.dram_tensor("eb2", [K, 4096], F16, kind="ExternalInput")
    P["out_d"] = nc.dram_tensor("out", [RPC, Q, 256], F32,
                                kind="ExternalOutput")

    with TileContext(nc) as tc:
        with (
            tc.tile_pool(name="const", bufs=1) as cpool,
            tc.tile_pool(name="xin", bufs=2) as x_pool,
            tc.tile_pool(name="kt", bufs=2) as kt_pool,
            tc.tile_pool(name="qt", bufs=2) as qt_pool,
            tc.tile_pool(name="gt", bufs=2) as gt_pool,
            tc.tile_pool(name="vt", bufs=2) as vt_pool,
            tc.tile_pool(name="esb", bufs=2) as e_pool,
            tc.tile_pool(name="asb", bufs=2) as a_pool,
            tc.tile_pool(name="rcp", bufs=2) as r_pool,
            tc.tile_pool(name="otg", bufs=2) as og_pool,
            tc.tile_pool(name="osb", bufs=2) as o_pool,
            tc.tile_pool(name="pslt", bufs=1, space="PSUM") as ps_lt,
            tc.tile_pool(name="pskv", bufs=2, space="PSUM") as ps_kv,
            tc.tile_pool(name="psobf", bufs=2, space="PSUM") as ps_obf,
        ):
            for nm in ("q", "k", "v", "g", "o"):
                tiles = [cpool.tile([128, 256], F16, tag=f"w{nm}{c}",
                                    name=f"w{nm}{c}") for c in range(2)]
                for c in range(2):
                    nc.sync.dma_start(tiles[c][:],
                                      wd[nm][c * 128:(c + 1) * 128, :])
                P[f"w{nm}_sb"] = tiles
            b1_sb = cpool.tile([128, 2 * RPC], F32, tag="b1", name="b1")
            nc.sync.dma_start(b1_sb[:], b1_d[:])
            P["b1_sb"] = b1_sb
            eb2_sb = [cpool.tile([128, 4096], F16, tag=f"eb2{c}",
                                 name=f"eb2{c}") for c in range(2)]
            for c in range(2):
                nc.sync.dma_start(eb2_sb[c][:], eb2_d[c * 128:(c + 1) * 128, :])
            P["eb2_sb"] = eb2_sb
            two_sb = cpool.tile([128, 32], F16, tag="two", name="two")
            nc.vector.memset(two_sb[:], 2.0)
            P["two_sb"] = two_sb

            P.update({"xin": x_pool, "kt": kt_pool, "qt": qt_pool,
                      "gt": gt_pool, "vt": vt_pool, "esb": e_pool,
                      "asb": a_pool, "rcp": r_pool, "otg": og_pool,
                      "osb": o_pool, "pslt": ps_lt, "pskv": ps_kv,
                      "psobf": ps_obf})

            # software pipeline: proj(p+1) sits between logits(p) and
            # back(p) in every engine queue, so the PE has independent
            # work while ACT runs pair p's exp chain.
            ctxs = [None] * PAIRS
            ctxs[0] = _emit_proj(nc, tc, P, 0)
            _emit_attn(nc, tc, P, 0, ctxs[0])
            for p in range(PAIRS):
                if p + 1 < PAIRS:
                    ctxs[p + 1] = _emit_proj(nc, tc, P, p + 1)
                _emit_back(nc, tc, P, p, ctxs[p])
                if p + 1 < PAIRS:
                    _emit_attn(nc, tc, P, p + 1, ctxs[p + 1])

    nc.compile()
    return nc


def host_prep(q_x, kv_x, bias1, bias2, wq, wk, wv, wg, wo):
    wqt = np.ascontiguousarray((wq / math.sqrt(CH)).T.astype(np.float16))
    wkt = np.ascontiguousarray(wk.T.astype(np.float16))
    wvt = np.ascontiguousarray(wv.T.astype(np.float16))
    wgt = np.ascontiguousarray(wg.T.astype(np.float16))
    wot = np.ascontiguousarray(wo.T.astype(np.float16))

    b2 = bias2[0, 0]
    eb2 = np.exp(b2.astype(np.float32)).transpose(2, 0, 1)   # [K, H, Q]
    eb2 = eb2.reshape(K, 2, 4, Q)
    eb2 = np.broadcast_to(eb2[:, :, None, :, :], (K, 2, 2, 4, Q))
    eb2 = np.ascontiguousarray(eb2.reshape(K, 4096).astype(np.float16))

    in_maps = []
    for c in range(NCORES):
        rows = slice(c * RPC, (c + 1) * RPC)
        qx = q_x[0, rows]
        qxp = qx.reshape(PAIRS, 2, Q, CQ).transpose(0, 3, 1, 2)
        qxp = qxp.reshape(PAIRS, CQ, 512)
        kvx = kv_x[0, rows]
        kvp = kvx.reshape(PAIRS, 2, K, CQ).transpose(0, 3, 1, 2)
        kvp = kvp.reshape(PAIRS, CQ, 512)
        xin = np.concatenate([kvp, qxp], axis=1).astype(np.float16)
        xin = xin.reshape(PAIRS, 4, 128, 512).transpose(0, 2, 1, 3)
        xin = np.ascontiguousarray(xin.reshape(PAIRS, 128, 2048))
        b1 = bias1[0, rows, 0, 0, :].astype(np.float32) - 4.0
        b1s = np.ascontiguousarray(
            b1.reshape(RPC, 2, 128).transpose(2, 0, 1).reshape(128, 2 * RPC))
        in_maps.append({
            "xin": xin, "wqt": wqt, "wkt": wkt, "wvt": wvt,
            "wgt": wgt, "wot": wot, "b1s": b1s, "eb2": eb2,
        })
    return in_maps


def gather(results):
    out = np.empty((1, S, Q, CQ), dtype=np.float32)
    for c in range(NCORES):
        out[0, c * RPC:(c + 1) * RPC] = results[c]["out"]
    return out


_NC_CACHE = None


def kernel_traced(q_x, kv_x, bias1, bias2, wq, wk, wv, wg, wo, trace=False):
    """Returns (full output [1,128,256,256] fp32, BassKernelResults)."""
    from concourse.bass_utils import run_bass_kernel_spmd
    global _NC_CACHE
    if _NC_CACHE is None:
        _NC_CACHE = build_nc()
    q_x, kv_x = np.asarray(q_x), np.asarray(kv_x)
    bias1, bias2 = np.asarray(bias1), np.asarray(bias2)
    wq, wk, wv, wg, wo = (np.asarray(w) for w in (wq, wk, wv, wg, wo))
    in_maps = host_prep(q_x, kv_x, bias1, bias2, wq, wk, wv, wg, wo)
    res = run_bass_kernel_spmd(_NC_CACHE, in_maps, list(range(NCORES)),
                               trace=trace)
    return gather(res.results), res


def kernel(q_x, kv_x, bias1, bias2, wq, wk, wv, wg, wo):
    """Full (unsharded) inputs in, full output out. Shards the 128 MSA
    rows across the 8 NeuronCores internally."""
    out, _ = kernel_traced(q_x, kv_x, bias1, bias2, wq, wk, wv, wg, wo)
    return out
